# revision 44
# baseline (speedup 1.0000x reference)
"""Trainium2 Bass kernel for nn_EvolutionCrossAttention (B=4, C=128, N=32*64*64).

8-core SPMD, sequence(N)-sharded. Per (b,h) the module reduces to
    logits[n] = const + sum_c Rf[c,bh] * x[b,c,n],   Rf = m' * rstd_group
    out       = f( sum_n softmax_n(logits) * x[b,:,n] )
with m' folding q@Wk, the GroupNorm affine and attn scale (host), and f the
tiny O(C^2) output projections (host). Softmax constants cancel in s/Z, so a
fixed -5*ln2 shift inside exp keeps p in fp8 range; no max pass needed.

Device kernel per core (all x traffic in fp8e4m3, two host-prepared layouts,
bulk DMA fanned out over the SP/ACT HWDGE queues + Pool SWDGE queue):
  xt [b, p, j*C+c] = x[b, c, j*128+p]  (n-partitioned: pool + stats)
  xn [b, c, n]                          (C-partitioned: logits)
  phase A: DMA xt; per b a DoubleRow fp8 gram on PE accumulates x@x.T in PSUM
           (diag = channel sumsq, extracted by mult-with-identity + reduce)
           plus a ones-matmul for channel sums; out free dims are tiny so PE
           cost is negligible under the v2 cost model.
  stats:   per-channel (sums|sumsq) partials -> 4KB AllGather -> strided-AP
           reduce -> group matmul -> rstd = exp(-0.5*ln(var+eps)) (stays on
           the preloaded natural_log_exp ACT table; Sqrt would force a
           second table load) -> Rf bf16 via one stride-0-broadcast mult.
  tail:    per 32-chunk group: logits = xn_chunk.T @ Rf (PE stationary=xn,
           moving=Rf, out free 4), p = exp(logits - 5ln2) -> fp8 (one ACT op
           per group), pool s += xt_pair.T @ p_pair and Z += ones.T @ p_pair
           as DoubleRow fp8 matmuls (out free 4/16). s, Z, ar merged on host.
Host merges (s, Z, channel stats) across cores and applies GroupNorm affine +
Wv/Wo in f64.
"""
import sys

sys.path.insert(0, "/opt/trn_rl_repo")

import numpy as np
import ml_dtypes

import concourse.bass as bass
import concourse.tile as tile
from concourse import mybir
from concourse.bass_utils import run_bass_kernel_spmd

# Problem dims (hardcoded per spec)
B, C = 4, 128
N = 32 * 64 * 64          # 131072
E = 128
NH, HD = 4, 32            # heads, head dim
G, GS = 8, 16             # groupnorm groups, channels per group
EPS = 1e-5
NCORES = 8
NS = N // NCORES          # 16384 per-core columns
NCH = NS // 128           # 128 chunks of 128 positions
BH = B * NH               # 16
NGRP = 4                  # tail groups
GCH = NCH // NGRP         # 16 chunks per group
EXP_SHIFT = -5.0 * float(np.log(2.0))

F32 = mybir.dt.float32
BF16 = mybir.dt.bfloat16
FP8 = mybir.dt.float8e4
DR = mybir.MatmulPerfMode.DoubleRow

_ISA_WAIT_LIMIT = 1


def _split_excess_waits(nc, limit=_ISA_WAIT_LIMIT):
    """This toolchain's codegen accepts only one sem wait per instruction;
    hoist extras onto same-engine nops inserted just before."""
    for bb in nc.main_func.blocks:
        insts = bb.instructions
        i = 0
        while i < len(insts):
            inst = insts[i]
            si = inst.sync_info
            if si is None or not si.on_wait or len(si.on_wait) <= limit:
                i += 1
                continue
            waits = list(si.on_wait)
            si.on_wait = waits[:limit]
            excess = waits[limit:]
            pos = i
            while excess:
                chunk, excess = excess[:limit], excess[limit:]
                nop = mybir.InstNoOp(name=nc.get_next_instruction_name(), ins=[], outs=[])
                nop.engine = inst.engine
                nop.sync_info = mybir.SyncInfo(on_wait=chunk, on_update=[])
                insts.insert(pos, nop)
                pos += 1
                i += 1
            i += 1


def _build_nc(ncores=NCORES, waitfix=True):
    nc = bass.Bass()
    xt = nc.declare_dram_parameter("xt", [B, 128, NS], FP8, isOutput=False)
    xn = nc.declare_dram_parameter("xn", [B, C, NS], FP8, isOutput=False)
    m2 = nc.declare_dram_parameter("m2", [C, BH], F32, isOutput=False)
    gind = nc.declare_dram_parameter("gind", [C, G], F32, isOutput=False)
    gindT = nc.declare_dram_parameter("gindT", [G, C], F32, isOutput=False)
    identC = nc.declare_dram_parameter("identC", [C, C], F32, isOutput=False)
    sout = nc.declare_dram_parameter("sout", [C, BH], F32, isOutput=True)
    zout = nc.declare_dram_parameter("zout", [1, BH], F32, isOutput=True)
    arout = nc.declare_dram_parameter("arout", [C, 2 * B], F32, isOutput=True)

    with tile.TileContext(nc) as tc:
        from contextlib import ExitStack
        with ExitStack() as ctx:
            consts = ctx.enter_context(tc.tile_pool(name="consts", bufs=1))
            small = ctx.enter_context(tc.tile_pool(name="small", bufs=1))
            p8pool = ctx.enter_context(tc.tile_pool(name="p8p", bufs=2))
            ptp = ctx.enter_context(tc.tile_pool(name="ptp", bufs=2, space="PSUM"))
            gramp = ctx.enter_context(tc.tile_pool(name="gramp", bufs=1, space="PSUM"))
            accp = ctx.enter_context(tc.tile_pool(name="accp", bufs=1, space="PSUM"))
            mmp = ctx.enter_context(tc.tile_pool(name="mmp", bufs=1, space="PSUM"))
            dram = ctx.enter_context(tc.tile_pool(name="dram", bufs=1, space="DRAM"))

            # ---- constants (gpsimd DMA queue; SP queue reserved for bulk x) ----
            m2_sb = consts.tile([C, BH], F32)
            nc.gpsimd.dma_start(m2_sb[:], m2[:])
            gind_sb = consts.tile([C, G], F32)
            nc.gpsimd.dma_start(gind_sb[:], gind[:])
            gindT_sb = consts.tile([G, C], F32)
            nc.gpsimd.dma_start(gindT_sb[:], gindT[:])
            identC_sb = consts.tile([C, C], F32)
            nc.gpsimd.dma_start(identC_sb[:], identC[:])
            # pair-stride must be >=16B and even for dual-fp8 ldweights
            ones8 = consts.tile([128, 2, 16], FP8, tag="ones8")
            nc.vector.memset(ones8[:], 1.0)
            ebias = consts.tile([128, 1], F32, tag="ebias")
            nc.vector.memset(ebias[:], EXP_SHIFT)
            c_inv = consts.tile([G, 1], F32, tag="cinv")
            nc.vector.memset(c_inv[:], 1.0 / (GS * NS * ncores))
            c_eps = consts.tile([G, 1], F32, tag="ceps")
            nc.vector.memset(c_eps[:], float(EPS))

            # ---- bulk x DMAs on SP queue: all xt parts first, then xn ----
            xt_sb = []
            dmaq = [nc.sync, nc.scalar, nc.gpsimd]
            qi = 0
            xt_bounds = [16 * k for k in range(8)] + [120, NCH]
            for b in range(B):
                t = consts.tile([128, NCH, C], FP8, name=f"xt{b}", tag=f"xt{b}")
                for lo, hi in zip(xt_bounds[:-1], xt_bounds[1:]):
                    dmaq[qi % 3].dma_start(
                        t[:, lo:hi, :], xt[b, :, lo * 128:hi * 128])
                    qi += 1
                xt_sb.append(t)
            xn_sb = [consts.tile([C, NS], FP8, name=f"xn{b}", tag=f"xn{b}")
                     for b in range(B)]
            for q in range(8):
                for b in range(B):
                    dmaq[qi % 2].dma_start(
                        xn_sb[b][:, q * (NS // 8):(q + 1) * (NS // 8)],
                        xn[b, :, q * (NS // 8):(q + 1) * (NS // 8)])
                    qi += 1

            # remaining consts + ACT table preload, behind the bulk shares on
            # their queues (all consumed only after the collective returns)
            m2_sb = consts.tile([C, BH], F32)
            nc.gpsimd.dma_start(m2_sb[:], m2[:])
            gind_sb = consts.tile([C, G], F32)
            nc.gpsimd.dma_start(gind_sb[:], gind[:])
            gindT_sb = consts.tile([G, C], F32)
            nc.gpsimd.dma_start(gindT_sb[:], gindT[:])
            # preload the natural_log_exp ACT table (Sqrt would force a second
            # table; rstd uses exp(-0.5*ln(v)))
            dum = consts.tile([G, 1], F32, tag="dum")
            nc.scalar.activation(dum[:], c_eps[:],
                                 mybir.ActivationFunctionType.Ln)
            dum2 = consts.tile([G, 1], F32, tag="dum2")
            nc.scalar.activation(dum2[:], dum[:],
                                 mybir.ActivationFunctionType.Exp)

            # ---- phase A: per-b gram (sumsq via diag) + channel sums on PE ----
            gall = gramp.tile([C, B, C], F32, tag="gall")
            gram_ps = [gall[:, b, :] for b in range(B)]
            sums_t = accp.tile([C, B], F32, tag="sums")
            sums_ps = sums_t[:]
            szp_t = accp.tile([C, BH], F32, tag="szp")
            szp = szp_t[:]
            ar_sb = small.tile([C, 2 * B], F32, tag="ar")
            for b in range(B):
                for i in range(NCH // 2):
                    sl = xt_sb[b][:, 2 * i:2 * i + 2, :]
                    nc.tensor.matmul(gram_ps[b], sl, sl,
                                     start=(i == 0), stop=(i == NCH // 2 - 1),
                                     perf_mode=DR)
                    nc.tensor.matmul(sums_ps[:, b:b + 1], sl, ones8[:, :, 0:1],
                                     start=(i == 0), stop=(i == NCH // 2 - 1),
                                     perf_mode=DR)
                # per-b stats extraction right away (DVE runs it while later
                # b's grams are still streaming)
                nc.vector.tensor_copy(ar_sb[:, b:b + 1], sums_ps[:, b:b + 1])
                dt_ = small.tile([C, C], F32, tag=f"dtmp{b}", name=f"dt{b}")
                nc.vector.scalar_tensor_tensor(
                    dt_[:], gram_ps[b], 1.0, identC_sb[:],
                    op0=mybir.AluOpType.mult, op1=mybir.AluOpType.mult,
                    accum_out=ar_sb[:, B + b:B + b + 1])

            # ---- cross-core exchange: AllGather + local sum ----
            ar_in = dram.tile([C, 2 * B], F32, tag="arin")
            ar_out = dram.tile([ncores, C, 2 * B], F32, tag="arout_d")
            nc.gpsimd.dma_start(ar_in[:], ar_sb[:])
            nc.gpsimd.collective_compute(
                "AllGather", mybir.AluOpType.bypass,
                replica_groups=[list(range(ncores))],
                ins=[ar_in.opt()], outs=[ar_out.opt()],
            )
            ag = small.tile([C, ncores, 2 * B], F32, tag="ag")
            nc.gpsimd.dma_start(ag[:], ar_out[:].rearrange("k c s -> c k s"))
            nc.scalar.dma_start(arout[:], ar_sb[:])
            cur = small.tile([C, 2 * B], F32, tag="agred")
            if ncores > 1:
                # view free dims as (s, k), reduce innermost (k)
                agv = bass.AP(tensor=ag[:].tensor, offset=ag[:].offset,
                              ap=[list(ag[:].ap[0]), [1, 2 * B], [2 * B, ncores]])
                nc.vector.reduce_sum(cur[:], agv, axis=mybir.AxisListType.X)
            else:
                nc.vector.tensor_copy(cur[:], ag[:, 0, :])

            # ---- group stats -> rstd -> Rf (bf16) ----
            o8_ps = mmp.tile([G, 2 * B], F32, tag="mm")
            nc.tensor.matmul(o8_ps[:], gind_sb[:], cur[:], start=True, stop=True)
            mex = small.tile([G, 2 * B], F32, tag="mex")
            nc.vector.tensor_scalar_mul(mex[:], o8_ps[:], c_inv[:, 0:1])
            msq = small.tile([G, B], F32, tag="msq")
            nc.vector.tensor_mul(msq[:], mex[:, 0:B], mex[:, 0:B])
            var = small.tile([G, B], F32, tag="var")
            nc.vector.tensor_sub(var[:], mex[:, B:2 * B], msq[:])
            lnv = small.tile([G, B], F32, tag="lnv")
            nc.scalar.activation(lnv[:], var[:],
                                 mybir.ActivationFunctionType.Ln, bias=c_eps[:])
            rstd = small.tile([G, B], F32, tag="rstd")
            nc.scalar.activation(rstd[:], lnv[:],
                                 mybir.ActivationFunctionType.Exp, scale=-0.5)

            rb_ps = mmp.tile([C, B], F32, tag="mm")
            nc.tensor.matmul(rb_ps[:], gindT_sb[:], rstd[:], start=True, stop=True)
            rf_bf = small.tile([C, BH], BF16, tag="rfbf")
            rbv = bass.AP(tensor=rb_ps[:].tensor, offset=rb_ps[:].offset,
                          ap=[list(rb_ps[:].ap[0]), [1, B], [0, NH]])
            nc.vector.tensor_mul(rf_bf[:], m2_sb[:], rbv)

            # ---- tail: logits -> exp -> pool/Z per 16-chunk group ----
            zp = accp.tile([1, BH], F32, tag="zp")
            for grp in range(NGRP):
                pt = ptp.tile([128, B, GCH, NH], F32, tag="pt")
                for cc in range(GCH):
                    jj = grp * GCH + cc
                    for b in range(B):
                        nc.tensor.matmul(
                            pt[:, b, cc, :],
                            xn_sb[b][:, jj * 128:(jj + 1) * 128],
                            rf_bf[:, NH * b:NH * (b + 1)],
                            start=True, stop=True)
                p8 = p8pool.tile([128, B, GCH, NH], FP8, tag="p8")
                nc.scalar.activation(p8[:], pt[:],
                                     mybir.ActivationFunctionType.Exp,
                                     bias=ebias[:])
                for i in range(GCH // 2):
                    for b in range(B):
                        first = (grp == 0 and i == 0 and b == 0)
                        last = (grp == NGRP - 1 and i == GCH // 2 - 1
                                and b == B - 1)
                        nc.tensor.matmul(
                            zp[:, NH * b:NH * (b + 1)], ones8[:, :, 0:1],
                            p8[:, b, 2 * i:2 * i + 2, :],
                            start=first, stop=last, perf_mode=DR)
                        nc.tensor.matmul(
                            szp[:, NH * b:NH * (b + 1)],
                            xt_sb[b][:, grp * GCH + 2 * i:grp * GCH + 2 * i + 2, :],
                            p8[:, b, 2 * i:2 * i + 2, :],
                            start=first, stop=last, perf_mode=DR)

            s_sb = small.tile([C, BH], F32, tag="ssb")
            nc.vector.tensor_copy(s_sb[:], szp)
            nc.gpsimd.dma_start(sout[:], s_sb[:])
            z_sb = small.tile([1, BH], F32, tag="zsb")
            nc.vector.tensor_copy(z_sb[:], zp[:])
            nc.scalar.dma_start(zout[:], z_sb[:])

    if waitfix:
        _split_excess_waits(nc)
    return nc


_NC_CACHE = {}


def _get_nc():
    if "nc" not in _NC_CACHE:
        _NC_CACHE["nc"] = _build_nc()
    return _NC_CACHE["nc"]


def _host_prep(evolution_feat, ln_g, ln_b, gn_g, Wq, bq, Wk):
    """Everything O(C^2): layernorm, q, fold q@Wk with GN affine + attn scale."""
    e = evolution_feat.astype(np.float64)
    mu = e.mean(axis=-1, keepdims=True)
    var = e.var(axis=-1, keepdims=True)
    e = (e - mu) / np.sqrt(var + EPS) * ln_g.astype(np.float64) + ln_b.astype(np.float64)
    q = e @ Wq.T.astype(np.float64) + bq.astype(np.float64)      # (B, C)
    q = q.reshape(B, NH, HD)
    # M[b,h,c] = sum_d q[b,h,d] Wk[h*HD+d, c]
    Wkr = Wk.astype(np.float64).reshape(NH, HD, C)
    M = np.einsum("bhd,hdc->bhc", q, Wkr)
    Mfold = M * gn_g.astype(np.float64)[None, None, :] * (HD ** -0.5)
    m2v = np.ascontiguousarray(
        Mfold.transpose(2, 0, 1).reshape(C, BH)).astype(np.float32)
    cg = np.arange(C) // GS
    gindm = (cg[:, None] == np.arange(G)[None, :]).astype(np.float32)
    return m2v, gindm


def _make_core_inputs(x8, m2v, gindm, identv, core):
    sl = slice(core * NS, (core + 1) * NS)
    xns = np.ascontiguousarray(x8[:, :, sl])
    xts = np.ascontiguousarray(
        x8[:, :, sl].reshape(B, C, NCH, 128).transpose(0, 3, 2, 1)
        .reshape(B, 128, NS))
    return {"xt": xts, "xn": xns, "m2": m2v, "gind": gindm,
            "gindT": np.ascontiguousarray(gindm.T), "identC": identv,
            }


def kernel(diff_spatial, evolution_feat, ln_g, ln_b, gn_g, gn_b,
           Wq, bq, Wk, bk, Wv, bv, Wo, bo):
    nc = _get_nc()
    xfull = np.asarray(diff_spatial, np.float32).reshape(B, C, N)
    x8 = xfull.astype(ml_dtypes.float8_e4m3fn)

    m2v, gindm = _host_prep(
        np.asarray(evolution_feat, np.float32),
        np.asarray(ln_g, np.float32), np.asarray(ln_b, np.float32),
        np.asarray(gn_g, np.float32), np.asarray(Wq, np.float32),
        np.asarray(bq, np.float32), np.asarray(Wk, np.float32))
    identv = np.eye(C, dtype=np.float32)

    in_maps = [_make_core_inputs(x8, m2v, gindm, identv, i)
               for i in range(NCORES)]
    res = run_bass_kernel_spmd(nc, in_maps, list(range(NCORES)))
    global _LAST_RES
    _LAST_RES = res
    return _host_finish(res.results, gn_g, gn_b, Wv, bv, Wo, bo)


_LAST_RES = None


def _host_finish(results, gn_g, gn_b, Wv, bv, Wo, bo):
    s_tot = np.zeros((C, BH), np.float64)
    z_tot = np.zeros((1, BH), np.float64)
    ar_tot = np.zeros((C, 2 * B), np.float64)
    for r in results:
        s_tot += r["sout"].astype(np.float64)
        z_tot += r["zout"].astype(np.float64)
        ar_tot += r["arout"].astype(np.float64)

    cg = np.arange(C) // GS
    gsel = cg[:, None] == np.arange(G)[None, :]                  # (C, G)
    sums = ar_tot[:, 0:B]                                        # (C, B)
    sumsq = ar_tot[:, B:2 * B]
    cnt = GS * N
    mean_g = (gsel.T.astype(np.float64) @ sums).T / cnt          # (B, G)
    ex2_g = (gsel.T.astype(np.float64) @ sumsq).T / cnt
    var_g = ex2_g - mean_g ** 2
    r_g = 1.0 / np.sqrt(var_g + EPS)

    a = r_g[:, cg] * np.asarray(gn_g, np.float64)[None, :]       # (B, C)
    d = np.asarray(gn_b, np.float64)[None, :] - mean_g[:, cg] * a
    sv = s_tot.reshape(C, B, NH).transpose(1, 2, 0)              # (B, NH, C)
    zv = z_tot.reshape(B, NH)
    y = a[:, None, :] * (sv / zv[:, :, None]) + d[:, None, :]    # (B, NH, C)

    Wvr = np.asarray(Wv, np.float64).reshape(NH, HD, C)
    o1 = np.einsum("hdc,bhc->bhd", Wvr, y).reshape(B, C) + np.asarray(bv, np.float64)
    out = o1 @ np.asarray(Wo, np.float64).T + np.asarray(bo, np.float64)
    return out.astype(np.float32)


# revision 45
# speedup vs baseline: 1.0047x; 1.0047x over previous
"""Trainium2 Bass kernel for nn_EvolutionCrossAttention (B=4, C=128, N=32*64*64).

8-core SPMD, sequence(N)-sharded. Per (b,h) the module reduces to
    logits[n] = const + sum_c Rf[c,bh] * x[b,c,n],   Rf = m' * rstd_group
    out       = f( sum_n softmax_n(logits) * x[b,:,n] )
with m' folding q@Wk, the GroupNorm affine and attn scale (host), and f the
tiny O(C^2) output projections (host). Softmax constants cancel in s/Z, so a
fixed -5*ln2 shift inside exp keeps p in fp8 range; no max pass needed.

Device kernel per core (all x traffic in fp8e4m3, two host-prepared layouts,
bulk DMA fanned out over the SP/ACT HWDGE queues + Pool SWDGE queue):
  xt [b, p, j*C+c] = x[b, c, j*128+p]  (n-partitioned: pool + stats)
  xn [b, c, n]                          (C-partitioned: logits)
  phase A: DMA xt; per b a DoubleRow fp8 gram on PE accumulates x@x.T in PSUM
           (diag = channel sumsq, extracted by mult-with-identity + reduce)
           plus a ones-matmul for channel sums; out free dims are tiny so PE
           cost is negligible under the v2 cost model.
  stats:   per-channel (sums|sumsq) partials -> 4KB AllGather -> strided-AP
           reduce -> group matmul -> rstd = exp(-0.5*ln(var+eps)) (stays on
           the preloaded natural_log_exp ACT table; Sqrt would force a
           second table load) -> Rf bf16 via one stride-0-broadcast mult.
  tail:    per 32-chunk group: logits = xn_chunk.T @ Rf (PE stationary=xn,
           moving=Rf, out free 4), p = exp(logits - 5ln2) -> fp8 (one ACT op
           per group), pool s += xt_pair.T @ p_pair and Z += ones.T @ p_pair
           as DoubleRow fp8 matmuls (out free 4/16). s, Z, ar merged on host.
Host merges (s, Z, channel stats) across cores and applies GroupNorm affine +
Wv/Wo in f64.
"""
import sys

sys.path.insert(0, "/opt/trn_rl_repo")

import numpy as np
import ml_dtypes

import concourse.bass as bass
import concourse.tile as tile
from concourse import mybir
from concourse.bass_utils import run_bass_kernel_spmd

# Problem dims (hardcoded per spec)
B, C = 4, 128
N = 32 * 64 * 64          # 131072
E = 128
NH, HD = 4, 32            # heads, head dim
G, GS = 8, 16             # groupnorm groups, channels per group
EPS = 1e-5
NCORES = 8
NS = N // NCORES          # 16384 per-core columns
NCH = NS // 128           # 128 chunks of 128 positions
BH = B * NH               # 16
NGRP = 4                  # tail groups
GCH = NCH // NGRP         # 16 chunks per group
EXP_SHIFT = -5.0 * float(np.log(2.0))

F32 = mybir.dt.float32
BF16 = mybir.dt.bfloat16
FP8 = mybir.dt.float8e4
DR = mybir.MatmulPerfMode.DoubleRow

_ISA_WAIT_LIMIT = 1


def _split_excess_waits(nc, limit=_ISA_WAIT_LIMIT):
    """This toolchain's codegen accepts only one sem wait per instruction;
    hoist extras onto same-engine nops inserted just before."""
    for bb in nc.main_func.blocks:
        insts = bb.instructions
        i = 0
        while i < len(insts):
            inst = insts[i]
            si = inst.sync_info
            if si is None or not si.on_wait or len(si.on_wait) <= limit:
                i += 1
                continue
            waits = list(si.on_wait)
            si.on_wait = waits[:limit]
            excess = waits[limit:]
            pos = i
            while excess:
                chunk, excess = excess[:limit], excess[limit:]
                nop = mybir.InstNoOp(name=nc.get_next_instruction_name(), ins=[], outs=[])
                nop.engine = inst.engine
                nop.sync_info = mybir.SyncInfo(on_wait=chunk, on_update=[])
                insts.insert(pos, nop)
                pos += 1
                i += 1
            i += 1


def _build_nc(ncores=NCORES, waitfix=True):
    nc = bass.Bass()
    xt = nc.declare_dram_parameter("xt", [B, 128, NS], FP8, isOutput=False)
    xn = nc.declare_dram_parameter("xn", [B, C, NS], FP8, isOutput=False)
    m2 = nc.declare_dram_parameter("m2", [C, BH], F32, isOutput=False)
    gind = nc.declare_dram_parameter("gind", [C, G], F32, isOutput=False)
    gindT = nc.declare_dram_parameter("gindT", [G, C], F32, isOutput=False)
    identC = nc.declare_dram_parameter("identC", [C, C], F32, isOutput=False)
    sout = nc.declare_dram_parameter("sout", [C, BH], F32, isOutput=True)
    zout = nc.declare_dram_parameter("zout", [1, BH], F32, isOutput=True)
    arout = nc.declare_dram_parameter("arout", [C, 2 * B], F32, isOutput=True)

    with tile.TileContext(nc) as tc:
        from contextlib import ExitStack
        with ExitStack() as ctx:
            consts = ctx.enter_context(tc.tile_pool(name="consts", bufs=1))
            small = ctx.enter_context(tc.tile_pool(name="small", bufs=1))
            p8pool = ctx.enter_context(tc.tile_pool(name="p8p", bufs=2))
            ptp = ctx.enter_context(tc.tile_pool(name="ptp", bufs=2, space="PSUM"))
            gramp = ctx.enter_context(tc.tile_pool(name="gramp", bufs=1, space="PSUM"))
            accp = ctx.enter_context(tc.tile_pool(name="accp", bufs=1, space="PSUM"))
            mmp = ctx.enter_context(tc.tile_pool(name="mmp", bufs=1, space="PSUM"))
            dram = ctx.enter_context(tc.tile_pool(name="dram", bufs=1, space="DRAM"))

            # ---- constants (gpsimd DMA queue; SP queue reserved for bulk x) ----
            m2_sb = consts.tile([C, BH], F32)
            nc.gpsimd.dma_start(m2_sb[:], m2[:])
            gind_sb = consts.tile([C, G], F32)
            nc.gpsimd.dma_start(gind_sb[:], gind[:])
            gindT_sb = consts.tile([G, C], F32)
            nc.gpsimd.dma_start(gindT_sb[:], gindT[:])
            identC_sb = consts.tile([C, C], F32)
            nc.gpsimd.dma_start(identC_sb[:], identC[:])
            # pair-stride must be >=16B and even for dual-fp8 ldweights
            ones8 = consts.tile([128, 2, 16], FP8, tag="ones8")
            nc.vector.memset(ones8[:], 1.0)
            ebias = consts.tile([128, 1], F32, tag="ebias")
            nc.vector.memset(ebias[:], EXP_SHIFT)
            c_inv = consts.tile([G, 1], F32, tag="cinv")
            nc.vector.memset(c_inv[:], 1.0 / (GS * NS * ncores))
            c_eps = consts.tile([G, 1], F32, tag="ceps")
            nc.vector.memset(c_eps[:], float(EPS))

            # ---- bulk x DMAs on SP queue: all xt parts first, then xn ----
            xt_sb = []
            dmaq = [nc.sync, nc.scalar, nc.gpsimd]
            qi = 0
            for b in range(B):
                t = consts.tile([128, NCH, C], FP8, name=f"xt{b}", tag=f"xt{b}")
                for q in range(8):
                    dmaq[qi % 3].dma_start(
                        t[:, q * (NCH // 8):(q + 1) * (NCH // 8), :],
                        xt[b, :, q * (NS // 8):(q + 1) * (NS // 8)])
                    qi += 1
                xt_sb.append(t)
            xn_sb = [consts.tile([C, NS], FP8, name=f"xn{b}", tag=f"xn{b}")
                     for b in range(B)]
            for q in range(8):
                for b in range(B):
                    dmaq[qi % 2].dma_start(
                        xn_sb[b][:, q * (NS // 8):(q + 1) * (NS // 8)],
                        xn[b, :, q * (NS // 8):(q + 1) * (NS // 8)])
                    qi += 1

            # remaining consts + ACT table preload, behind the bulk shares on
            # their queues (all consumed only after the collective returns)
            m2_sb = consts.tile([C, BH], F32)
            nc.gpsimd.dma_start(m2_sb[:], m2[:])
            gind_sb = consts.tile([C, G], F32)
            nc.gpsimd.dma_start(gind_sb[:], gind[:])
            gindT_sb = consts.tile([G, C], F32)
            nc.gpsimd.dma_start(gindT_sb[:], gindT[:])
            # preload the natural_log_exp ACT table (Sqrt would force a second
            # table; rstd uses exp(-0.5*ln(v)))
            dum = consts.tile([G, 1], F32, tag="dum")
            nc.scalar.activation(dum[:], c_eps[:],
                                 mybir.ActivationFunctionType.Ln)
            dum2 = consts.tile([G, 1], F32, tag="dum2")
            nc.scalar.activation(dum2[:], dum[:],
                                 mybir.ActivationFunctionType.Exp)

            # ---- phase A: per-b gram (sumsq via diag) + channel sums on PE ----
            gall = gramp.tile([C, B, C], F32, tag="gall")
            gram_ps = [gall[:, b, :] for b in range(B)]
            sums_t = accp.tile([C, B], F32, tag="sums")
            sums_ps = sums_t[:]
            szp_t = accp.tile([C, BH], F32, tag="szp")
            szp = szp_t[:]
            ar_sb = small.tile([C, 2 * B], F32, tag="ar")
            for b in range(B):
                for i in range(NCH // 2):
                    sl = xt_sb[b][:, 2 * i:2 * i + 2, :]
                    nc.tensor.matmul(gram_ps[b], sl, sl,
                                     start=(i == 0), stop=(i == NCH // 2 - 1),
                                     perf_mode=DR)
                    nc.tensor.matmul(sums_ps[:, b:b + 1], sl, ones8[:, :, 0:1],
                                     start=(i == 0), stop=(i == NCH // 2 - 1),
                                     perf_mode=DR)
                # per-b stats extraction right away (DVE runs it while later
                # b's grams are still streaming)
                nc.vector.tensor_copy(ar_sb[:, b:b + 1], sums_ps[:, b:b + 1])
                dt_ = small.tile([C, C], F32, tag=f"dtmp{b}", name=f"dt{b}")
                nc.vector.scalar_tensor_tensor(
                    dt_[:], gram_ps[b], 1.0, identC_sb[:],
                    op0=mybir.AluOpType.mult, op1=mybir.AluOpType.mult,
                    accum_out=ar_sb[:, B + b:B + b + 1])

            # ---- cross-core exchange: AllGather + local sum ----
            ar_in = dram.tile([C, 2 * B], F32, tag="arin")
            ar_out = dram.tile([ncores, C, 2 * B], F32, tag="arout_d")
            nc.gpsimd.dma_start(ar_in[:], ar_sb[:])
            nc.gpsimd.collective_compute(
                "AllGather", mybir.AluOpType.bypass,
                replica_groups=[list(range(ncores))],
                ins=[ar_in.opt()], outs=[ar_out.opt()],
            )
            ag = small.tile([C, ncores, 2 * B], F32, tag="ag")
            nc.gpsimd.dma_start(ag[:], ar_out[:].rearrange("k c s -> c k s"))
            nc.scalar.dma_start(arout[:], ar_sb[:])
            cur = small.tile([C, 2 * B], F32, tag="agred")
            if ncores > 1:
                # view free dims as (s, k), reduce innermost (k)
                agv = bass.AP(tensor=ag[:].tensor, offset=ag[:].offset,
                              ap=[list(ag[:].ap[0]), [1, 2 * B], [2 * B, ncores]])
                nc.vector.reduce_sum(cur[:], agv, axis=mybir.AxisListType.X)
            else:
                nc.vector.tensor_copy(cur[:], ag[:, 0, :])

            # ---- group stats -> rstd -> Rf (bf16) ----
            o8_ps = mmp.tile([G, 2 * B], F32, tag="mm")
            nc.tensor.matmul(o8_ps[:], gind_sb[:], cur[:], start=True, stop=True)
            mex = small.tile([G, 2 * B], F32, tag="mex")
            nc.vector.tensor_scalar_mul(mex[:], o8_ps[:], c_inv[:, 0:1])
            msq = small.tile([G, B], F32, tag="msq")
            nc.vector.tensor_mul(msq[:], mex[:, 0:B], mex[:, 0:B])
            var = small.tile([G, B], F32, tag="var")
            nc.vector.tensor_sub(var[:], mex[:, B:2 * B], msq[:])
            lnv = small.tile([G, B], F32, tag="lnv")
            nc.scalar.activation(lnv[:], var[:],
                                 mybir.ActivationFunctionType.Ln, bias=c_eps[:])
            rstd = small.tile([G, B], F32, tag="rstd")
            nc.scalar.activation(rstd[:], lnv[:],
                                 mybir.ActivationFunctionType.Exp, scale=-0.5)

            rb_ps = mmp.tile([C, B], F32, tag="mm")
            nc.tensor.matmul(rb_ps[:], gindT_sb[:], rstd[:], start=True, stop=True)
            rf_bf = small.tile([C, BH], BF16, tag="rfbf")
            rbv = bass.AP(tensor=rb_ps[:].tensor, offset=rb_ps[:].offset,
                          ap=[list(rb_ps[:].ap[0]), [1, B], [0, NH]])
            nc.vector.tensor_mul(rf_bf[:], m2_sb[:], rbv)

            # ---- tail: logits -> exp -> pool/Z per 16-chunk group ----
            zp = accp.tile([1, BH], F32, tag="zp")
            for grp in range(NGRP):
                pt = ptp.tile([128, B, GCH, NH], F32, tag="pt")
                for cc in range(GCH):
                    jj = grp * GCH + cc
                    for b in range(B):
                        nc.tensor.matmul(
                            pt[:, b, cc, :],
                            xn_sb[b][:, jj * 128:(jj + 1) * 128],
                            rf_bf[:, NH * b:NH * (b + 1)],
                            start=True, stop=True)
                p8 = p8pool.tile([128, B, GCH, NH], FP8, tag="p8")
                nc.scalar.activation(p8[:], pt[:],
                                     mybir.ActivationFunctionType.Exp,
                                     bias=ebias[:])
                for i in range(GCH // 2):
                    for b in range(B):
                        first = (grp == 0 and i == 0 and b == 0)
                        last = (grp == NGRP - 1 and i == GCH // 2 - 1
                                and b == B - 1)
                        nc.tensor.matmul(
                            zp[:, NH * b:NH * (b + 1)], ones8[:, :, 0:1],
                            p8[:, b, 2 * i:2 * i + 2, :],
                            start=first, stop=last, perf_mode=DR)
                        nc.tensor.matmul(
                            szp[:, NH * b:NH * (b + 1)],
                            xt_sb[b][:, grp * GCH + 2 * i:grp * GCH + 2 * i + 2, :],
                            p8[:, b, 2 * i:2 * i + 2, :],
                            start=first, stop=last, perf_mode=DR)

            s_sb = small.tile([C, BH], F32, tag="ssb")
            nc.vector.tensor_copy(s_sb[:], szp)
            nc.gpsimd.dma_start(sout[:], s_sb[:])
            z_sb = small.tile([1, BH], F32, tag="zsb")
            nc.vector.tensor_copy(z_sb[:], zp[:])
            nc.scalar.dma_start(zout[:], z_sb[:])

    if waitfix:
        _split_excess_waits(nc)
    return nc


_NC_CACHE = {}


def _get_nc():
    if "nc" not in _NC_CACHE:
        _NC_CACHE["nc"] = _build_nc()
    return _NC_CACHE["nc"]


def _host_prep(evolution_feat, ln_g, ln_b, gn_g, Wq, bq, Wk):
    """Everything O(C^2): layernorm, q, fold q@Wk with GN affine + attn scale."""
    e = evolution_feat.astype(np.float64)
    mu = e.mean(axis=-1, keepdims=True)
    var = e.var(axis=-1, keepdims=True)
    e = (e - mu) / np.sqrt(var + EPS) * ln_g.astype(np.float64) + ln_b.astype(np.float64)
    q = e @ Wq.T.astype(np.float64) + bq.astype(np.float64)      # (B, C)
    q = q.reshape(B, NH, HD)
    # M[b,h,c] = sum_d q[b,h,d] Wk[h*HD+d, c]
    Wkr = Wk.astype(np.float64).reshape(NH, HD, C)
    M = np.einsum("bhd,hdc->bhc", q, Wkr)
    Mfold = M * gn_g.astype(np.float64)[None, None, :] * (HD ** -0.5)
    m2v = np.ascontiguousarray(
        Mfold.transpose(2, 0, 1).reshape(C, BH)).astype(np.float32)
    cg = np.arange(C) // GS
    gindm = (cg[:, None] == np.arange(G)[None, :]).astype(np.float32)
    return m2v, gindm


def _make_core_inputs(x8, m2v, gindm, identv, core):
    sl = slice(core * NS, (core + 1) * NS)
    xns = np.ascontiguousarray(x8[:, :, sl])
    xts = np.ascontiguousarray(
        x8[:, :, sl].reshape(B, C, NCH, 128).transpose(0, 3, 2, 1)
        .reshape(B, 128, NS))
    return {"xt": xts, "xn": xns, "m2": m2v, "gind": gindm,
            "gindT": np.ascontiguousarray(gindm.T), "identC": identv,
            }


def kernel(diff_spatial, evolution_feat, ln_g, ln_b, gn_g, gn_b,
           Wq, bq, Wk, bk, Wv, bv, Wo, bo):
    nc = _get_nc()
    xfull = np.asarray(diff_spatial, np.float32).reshape(B, C, N)
    x8 = xfull.astype(ml_dtypes.float8_e4m3fn)

    m2v, gindm = _host_prep(
        np.asarray(evolution_feat, np.float32),
        np.asarray(ln_g, np.float32), np.asarray(ln_b, np.float32),
        np.asarray(gn_g, np.float32), np.asarray(Wq, np.float32),
        np.asarray(bq, np.float32), np.asarray(Wk, np.float32))
    identv = np.eye(C, dtype=np.float32)

    in_maps = [_make_core_inputs(x8, m2v, gindm, identv, i)
               for i in range(NCORES)]
    res = run_bass_kernel_spmd(nc, in_maps, list(range(NCORES)))
    global _LAST_RES
    _LAST_RES = res
    return _host_finish(res.results, gn_g, gn_b, Wv, bv, Wo, bo)


_LAST_RES = None


def _host_finish(results, gn_g, gn_b, Wv, bv, Wo, bo):
    s_tot = np.zeros((C, BH), np.float64)
    z_tot = np.zeros((1, BH), np.float64)
    ar_tot = np.zeros((C, 2 * B), np.float64)
    for r in results:
        s_tot += r["sout"].astype(np.float64)
        z_tot += r["zout"].astype(np.float64)
        ar_tot += r["arout"].astype(np.float64)

    cg = np.arange(C) // GS
    gsel = cg[:, None] == np.arange(G)[None, :]                  # (C, G)
    sums = ar_tot[:, 0:B]                                        # (C, B)
    sumsq = ar_tot[:, B:2 * B]
    cnt = GS * N
    mean_g = (gsel.T.astype(np.float64) @ sums).T / cnt          # (B, G)
    ex2_g = (gsel.T.astype(np.float64) @ sumsq).T / cnt
    var_g = ex2_g - mean_g ** 2
    r_g = 1.0 / np.sqrt(var_g + EPS)

    a = r_g[:, cg] * np.asarray(gn_g, np.float64)[None, :]       # (B, C)
    d = np.asarray(gn_b, np.float64)[None, :] - mean_g[:, cg] * a
    sv = s_tot.reshape(C, B, NH).transpose(1, 2, 0)              # (B, NH, C)
    zv = z_tot.reshape(B, NH)
    y = a[:, None, :] * (sv / zv[:, :, None]) + d[:, None, :]    # (B, NH, C)

    Wvr = np.asarray(Wv, np.float64).reshape(NH, HD, C)
    o1 = np.einsum("hdc,bhc->bhd", Wvr, y).reshape(B, C) + np.asarray(bv, np.float64)
    out = o1 @ np.asarray(Wo, np.float64).T + np.asarray(bo, np.float64)
    return out.astype(np.float32)


# revision 46
# speedup vs baseline: 1.0491x; 1.0442x over previous
"""Trainium2 Bass kernel for nn_EvolutionCrossAttention (B=4, C=128, N=32*64*64).

8-core SPMD, sequence(N)-sharded. Per (b,h) the module reduces to
    logits[n] = const + sum_c Rf[c,bh] * x[b,c,n],   Rf = m' * rstd_group
    out       = f( sum_n softmax_n(logits) * x[b,:,n] )
with m' folding q@Wk, the GroupNorm affine and attn scale (host), and f the
tiny O(C^2) output projections (host). Softmax constants cancel in s/Z, so a
fixed -5*ln2 shift inside exp keeps p in fp8 range; no max pass needed.

Device kernel per core (all x traffic in fp8e4m3, two host-prepared layouts,
bulk DMA fanned out over the SP/ACT HWDGE queues + Pool SWDGE queue):
  xt [b, p, j*C+c] = x[b, c, j*128+p]  (n-partitioned: pool + stats)
  xn [b, c, n]                          (C-partitioned: logits)
  phase A: DMA xt; per b a DoubleRow fp8 gram on PE accumulates x@x.T in PSUM
           (diag = channel sumsq, extracted by mult-with-identity + reduce)
           plus a ones-matmul for channel sums; out free dims are tiny so PE
           cost is negligible under the v2 cost model.
  stats:   per-channel (sums|sumsq) partials -> 4KB AllGather -> strided-AP
           reduce -> group matmul -> rstd = exp(-0.5*ln(var+eps)) (stays on
           the preloaded natural_log_exp ACT table; Sqrt would force a
           second table load) -> Rf bf16 via one stride-0-broadcast mult.
  tail:    per 32-chunk group: logits = xn_chunk.T @ Rf (PE stationary=xn,
           moving=Rf, out free 4), p = exp(logits - 5ln2) -> fp8 (one ACT op
           per group), pool s += xt_pair.T @ p_pair and Z += ones.T @ p_pair
           as DoubleRow fp8 matmuls (out free 4/16). s, Z, ar merged on host.
Host merges (s, Z, channel stats) across cores and applies GroupNorm affine +
Wv/Wo in f64.
"""
import sys

sys.path.insert(0, "/opt/trn_rl_repo")

import numpy as np
import ml_dtypes

import concourse.bass as bass
import concourse.tile as tile
from concourse import mybir
from concourse.bass_utils import run_bass_kernel_spmd

# Problem dims (hardcoded per spec)
B, C = 4, 128
N = 32 * 64 * 64          # 131072
E = 128
NH, HD = 4, 32            # heads, head dim
G, GS = 8, 16             # groupnorm groups, channels per group
EPS = 1e-5
NCORES = 8
NS = N // NCORES          # 16384 per-core columns
NCH = NS // 128           # 128 chunks of 128 positions
BH = B * NH               # 16
NGRP = 4                  # tail groups
GCH = NCH // NGRP         # 16 chunks per group
EXP_SHIFT = -5.0 * float(np.log(2.0))

F32 = mybir.dt.float32
BF16 = mybir.dt.bfloat16
FP8 = mybir.dt.float8e4
DR = mybir.MatmulPerfMode.DoubleRow

_ISA_WAIT_LIMIT = 1


def _split_excess_waits(nc, limit=_ISA_WAIT_LIMIT):
    """This toolchain's codegen accepts only one sem wait per instruction;
    hoist extras onto same-engine nops inserted just before."""
    for bb in nc.main_func.blocks:
        insts = bb.instructions
        i = 0
        while i < len(insts):
            inst = insts[i]
            si = inst.sync_info
            if si is None or not si.on_wait or len(si.on_wait) <= limit:
                i += 1
                continue
            waits = list(si.on_wait)
            si.on_wait = waits[:limit]
            excess = waits[limit:]
            pos = i
            while excess:
                chunk, excess = excess[:limit], excess[limit:]
                nop = mybir.InstNoOp(name=nc.get_next_instruction_name(), ins=[], outs=[])
                nop.engine = inst.engine
                nop.sync_info = mybir.SyncInfo(on_wait=chunk, on_update=[])
                insts.insert(pos, nop)
                pos += 1
                i += 1
            i += 1


def _build_nc(ncores=NCORES, waitfix=True):
    nc = bass.Bass()
    xt = nc.declare_dram_parameter("xt", [B, 128, NS], FP8, isOutput=False)
    xn = nc.declare_dram_parameter("xn", [B, C, NS], FP8, isOutput=False)
    m2 = nc.declare_dram_parameter("m2", [C, BH], F32, isOutput=False)
    gind = nc.declare_dram_parameter("gind", [C, G], F32, isOutput=False)
    gindT = nc.declare_dram_parameter("gindT", [G, C], F32, isOutput=False)
    identC = nc.declare_dram_parameter("identC", [C, C], F32, isOutput=False)
    sout = nc.declare_dram_parameter("sout", [C, BH], F32, isOutput=True)
    zout = nc.declare_dram_parameter("zout", [1, BH], F32, isOutput=True)
    arout = nc.declare_dram_parameter("arout", [C, 2 * B], F32, isOutput=True)

    with tile.TileContext(nc) as tc:
        from contextlib import ExitStack
        with ExitStack() as ctx:
            consts = ctx.enter_context(tc.tile_pool(name="consts", bufs=1))
            small = ctx.enter_context(tc.tile_pool(name="small", bufs=1))
            p8pool = ctx.enter_context(tc.tile_pool(name="p8p", bufs=2))
            ptp = ctx.enter_context(tc.tile_pool(name="ptp", bufs=2, space="PSUM"))
            gramp = ctx.enter_context(tc.tile_pool(name="gramp", bufs=1, space="PSUM"))
            accp = ctx.enter_context(tc.tile_pool(name="accp", bufs=1, space="PSUM"))
            mmp = ctx.enter_context(tc.tile_pool(name="mmp", bufs=1, space="PSUM"))
            dram = ctx.enter_context(tc.tile_pool(name="dram", bufs=1, space="DRAM"))

            # ---- early consts: only identC is needed during phase A ----
            identC_sb = consts.tile([C, C], F32)
            nc.gpsimd.dma_start(identC_sb[:], identC[:])
            # pair-stride must be >=16B and even for dual-fp8 ldweights
            ones8 = consts.tile([128, 2, 16], FP8, tag="ones8")
            nc.vector.memset(ones8[:], 1.0)
            ebias = consts.tile([128, 1], F32, tag="ebias")
            nc.vector.memset(ebias[:], EXP_SHIFT)
            c_inv = consts.tile([G, 1], F32, tag="cinv")
            nc.vector.memset(c_inv[:], 1.0 / (GS * NS * ncores))
            c_eps = consts.tile([G, 1], F32, tag="ceps")
            nc.vector.memset(c_eps[:], float(EPS))

            # ---- bulk x DMAs on SP queue: all xt parts first, then xn ----
            xt_sb = []
            dmaq = [nc.sync, nc.scalar, nc.gpsimd]
            qi = 0
            for b in range(B):
                t = consts.tile([128, NCH, C], FP8, name=f"xt{b}", tag=f"xt{b}")
                for q in range(8):
                    dmaq[qi % 3].dma_start(
                        t[:, q * (NCH // 8):(q + 1) * (NCH // 8), :],
                        xt[b, :, q * (NS // 8):(q + 1) * (NS // 8)])
                    qi += 1
                xt_sb.append(t)
            xn_sb = [consts.tile([C, NS], FP8, name=f"xn{b}", tag=f"xn{b}")
                     for b in range(B)]
            for q in range(8):
                for b in range(B):
                    dmaq[qi % 2].dma_start(
                        xn_sb[b][:, q * (NS // 8):(q + 1) * (NS // 8)],
                        xn[b, :, q * (NS // 8):(q + 1) * (NS // 8)])
                    qi += 1

            # remaining consts + ACT table preload, behind the bulk shares on
            # their queues (all consumed only after the collective returns)
            m2_sb = consts.tile([C, BH], F32)
            nc.gpsimd.dma_start(m2_sb[:], m2[:])
            gind_sb = consts.tile([C, G], F32)
            nc.gpsimd.dma_start(gind_sb[:], gind[:])
            gindT_sb = consts.tile([G, C], F32)
            nc.gpsimd.dma_start(gindT_sb[:], gindT[:])
            # preload the natural_log_exp ACT table (Sqrt would force a second
            # table; rstd uses exp(-0.5*ln(v)))
            dum = consts.tile([G, 1], F32, tag="dum")
            nc.scalar.activation(dum[:], c_eps[:],
                                 mybir.ActivationFunctionType.Ln)
            dum2 = consts.tile([G, 1], F32, tag="dum2")
            nc.scalar.activation(dum2[:], dum[:],
                                 mybir.ActivationFunctionType.Exp)

            # ---- phase A: per-b gram (sumsq via diag) + channel sums on PE ----
            gall = gramp.tile([C, B, C], F32, tag="gall")
            gram_ps = [gall[:, b, :] for b in range(B)]
            sums_t = accp.tile([C, B], F32, tag="sums")
            sums_ps = sums_t[:]
            szp_t = accp.tile([C, BH], F32, tag="szp")
            szp = szp_t[:]
            ar_sb = small.tile([C, 2 * B], F32, tag="ar")
            for b in range(B):
                for i in range(NCH // 2):
                    sl = xt_sb[b][:, 2 * i:2 * i + 2, :]
                    nc.tensor.matmul(gram_ps[b], sl, sl,
                                     start=(i == 0), stop=(i == NCH // 2 - 1),
                                     perf_mode=DR)
                    nc.tensor.matmul(sums_ps[:, b:b + 1], sl, ones8[:, :, 0:1],
                                     start=(i == 0), stop=(i == NCH // 2 - 1),
                                     perf_mode=DR)
                # per-b stats extraction right away (DVE runs it while later
                # b's grams are still streaming)
                nc.vector.tensor_copy(ar_sb[:, b:b + 1], sums_ps[:, b:b + 1])
                dt_ = small.tile([C, C], F32, tag=f"dtmp{b}", name=f"dt{b}")
                nc.vector.scalar_tensor_tensor(
                    dt_[:], gram_ps[b], 1.0, identC_sb[:],
                    op0=mybir.AluOpType.mult, op1=mybir.AluOpType.mult,
                    accum_out=ar_sb[:, B + b:B + b + 1])

            # ---- cross-core exchange: AllGather + local sum ----
            ar_in = dram.tile([C, 2 * B], F32, tag="arin")
            ar_out = dram.tile([ncores, C, 2 * B], F32, tag="arout_d")
            nc.gpsimd.dma_start(ar_in[:], ar_sb[:])
            nc.gpsimd.collective_compute(
                "AllGather", mybir.AluOpType.bypass,
                replica_groups=[list(range(ncores))],
                ins=[ar_in.opt()], outs=[ar_out.opt()],
            )
            ag = small.tile([C, ncores, 2 * B], F32, tag="ag")
            nc.gpsimd.dma_start(ag[:], ar_out[:].rearrange("k c s -> c k s"))
            nc.scalar.dma_start(arout[:], ar_sb[:])
            cur = small.tile([C, 2 * B], F32, tag="agred")
            if ncores > 1:
                # view free dims as (s, k), reduce innermost (k)
                agv = bass.AP(tensor=ag[:].tensor, offset=ag[:].offset,
                              ap=[list(ag[:].ap[0]), [1, 2 * B], [2 * B, ncores]])
                nc.vector.reduce_sum(cur[:], agv, axis=mybir.AxisListType.X)
            else:
                nc.vector.tensor_copy(cur[:], ag[:, 0, :])

            # ---- group stats -> rstd -> Rf (bf16) ----
            o8_ps = mmp.tile([G, 2 * B], F32, tag="mm")
            nc.tensor.matmul(o8_ps[:], gind_sb[:], cur[:], start=True, stop=True)
            mex = small.tile([G, 2 * B], F32, tag="mex")
            nc.vector.tensor_scalar_mul(mex[:], o8_ps[:], c_inv[:, 0:1])
            msq = small.tile([G, B], F32, tag="msq")
            nc.vector.tensor_mul(msq[:], mex[:, 0:B], mex[:, 0:B])
            var = small.tile([G, B], F32, tag="var")
            nc.vector.tensor_sub(var[:], mex[:, B:2 * B], msq[:])
            lnv = small.tile([G, B], F32, tag="lnv")
            nc.scalar.activation(lnv[:], var[:],
                                 mybir.ActivationFunctionType.Ln, bias=c_eps[:])
            rstd = small.tile([G, B], F32, tag="rstd")
            nc.scalar.activation(rstd[:], lnv[:],
                                 mybir.ActivationFunctionType.Exp, scale=-0.5)

            rb_ps = mmp.tile([C, B], F32, tag="mm")
            nc.tensor.matmul(rb_ps[:], gindT_sb[:], rstd[:], start=True, stop=True)
            rf_bf = small.tile([C, BH], BF16, tag="rfbf")
            rbv = bass.AP(tensor=rb_ps[:].tensor, offset=rb_ps[:].offset,
                          ap=[list(rb_ps[:].ap[0]), [1, B], [0, NH]])
            nc.vector.tensor_mul(rf_bf[:], m2_sb[:], rbv)

            # ---- tail: logits -> exp -> pool/Z per 16-chunk group ----
            zp = accp.tile([1, BH], F32, tag="zp")
            for grp in range(NGRP):
                pt = ptp.tile([128, B, GCH, NH], F32, tag="pt")
                for cc in range(GCH):
                    jj = grp * GCH + cc
                    for b in range(B):
                        nc.tensor.matmul(
                            pt[:, b, cc, :],
                            xn_sb[b][:, jj * 128:(jj + 1) * 128],
                            rf_bf[:, NH * b:NH * (b + 1)],
                            start=True, stop=True)
                p8 = p8pool.tile([128, B, GCH, NH], FP8, tag="p8")
                nc.scalar.activation(p8[:], pt[:],
                                     mybir.ActivationFunctionType.Exp,
                                     bias=ebias[:])
                for i in range(GCH // 2):
                    for b in range(B):
                        first = (grp == 0 and i == 0 and b == 0)
                        last = (grp == NGRP - 1 and i == GCH // 2 - 1
                                and b == B - 1)
                        nc.tensor.matmul(
                            zp[:, NH * b:NH * (b + 1)], ones8[:, :, 0:1],
                            p8[:, b, 2 * i:2 * i + 2, :],
                            start=first, stop=last, perf_mode=DR)
                        nc.tensor.matmul(
                            szp[:, NH * b:NH * (b + 1)],
                            xt_sb[b][:, grp * GCH + 2 * i:grp * GCH + 2 * i + 2, :],
                            p8[:, b, 2 * i:2 * i + 2, :],
                            start=first, stop=last, perf_mode=DR)

            s_sb = small.tile([C, BH], F32, tag="ssb")
            nc.vector.tensor_copy(s_sb[:], szp)
            nc.gpsimd.dma_start(sout[:], s_sb[:])
            z_sb = small.tile([1, BH], F32, tag="zsb")
            nc.vector.tensor_copy(z_sb[:], zp[:])
            nc.scalar.dma_start(zout[:], z_sb[:])

    if waitfix:
        _split_excess_waits(nc)
    return nc


_NC_CACHE = {}


def _get_nc():
    if "nc" not in _NC_CACHE:
        _NC_CACHE["nc"] = _build_nc()
    return _NC_CACHE["nc"]


def _host_prep(evolution_feat, ln_g, ln_b, gn_g, Wq, bq, Wk):
    """Everything O(C^2): layernorm, q, fold q@Wk with GN affine + attn scale."""
    e = evolution_feat.astype(np.float64)
    mu = e.mean(axis=-1, keepdims=True)
    var = e.var(axis=-1, keepdims=True)
    e = (e - mu) / np.sqrt(var + EPS) * ln_g.astype(np.float64) + ln_b.astype(np.float64)
    q = e @ Wq.T.astype(np.float64) + bq.astype(np.float64)      # (B, C)
    q = q.reshape(B, NH, HD)
    # M[b,h,c] = sum_d q[b,h,d] Wk[h*HD+d, c]
    Wkr = Wk.astype(np.float64).reshape(NH, HD, C)
    M = np.einsum("bhd,hdc->bhc", q, Wkr)
    Mfold = M * gn_g.astype(np.float64)[None, None, :] * (HD ** -0.5)
    m2v = np.ascontiguousarray(
        Mfold.transpose(2, 0, 1).reshape(C, BH)).astype(np.float32)
    cg = np.arange(C) // GS
    gindm = (cg[:, None] == np.arange(G)[None, :]).astype(np.float32)
    return m2v, gindm


def _make_core_inputs(x8, m2v, gindm, identv, core):
    sl = slice(core * NS, (core + 1) * NS)
    xns = np.ascontiguousarray(x8[:, :, sl])
    xts = np.ascontiguousarray(
        x8[:, :, sl].reshape(B, C, NCH, 128).transpose(0, 3, 2, 1)
        .reshape(B, 128, NS))
    return {"xt": xts, "xn": xns, "m2": m2v, "gind": gindm,
            "gindT": np.ascontiguousarray(gindm.T), "identC": identv,
            }


def kernel(diff_spatial, evolution_feat, ln_g, ln_b, gn_g, gn_b,
           Wq, bq, Wk, bk, Wv, bv, Wo, bo):
    nc = _get_nc()
    xfull = np.asarray(diff_spatial, np.float32).reshape(B, C, N)
    x8 = xfull.astype(ml_dtypes.float8_e4m3fn)

    m2v, gindm = _host_prep(
        np.asarray(evolution_feat, np.float32),
        np.asarray(ln_g, np.float32), np.asarray(ln_b, np.float32),
        np.asarray(gn_g, np.float32), np.asarray(Wq, np.float32),
        np.asarray(bq, np.float32), np.asarray(Wk, np.float32))
    identv = np.eye(C, dtype=np.float32)

    in_maps = [_make_core_inputs(x8, m2v, gindm, identv, i)
               for i in range(NCORES)]
    res = run_bass_kernel_spmd(nc, in_maps, list(range(NCORES)))
    global _LAST_RES
    _LAST_RES = res
    return _host_finish(res.results, gn_g, gn_b, Wv, bv, Wo, bo)


_LAST_RES = None


def _host_finish(results, gn_g, gn_b, Wv, bv, Wo, bo):
    s_tot = np.zeros((C, BH), np.float64)
    z_tot = np.zeros((1, BH), np.float64)
    ar_tot = np.zeros((C, 2 * B), np.float64)
    for r in results:
        s_tot += r["sout"].astype(np.float64)
        z_tot += r["zout"].astype(np.float64)
        ar_tot += r["arout"].astype(np.float64)

    cg = np.arange(C) // GS
    gsel = cg[:, None] == np.arange(G)[None, :]                  # (C, G)
    sums = ar_tot[:, 0:B]                                        # (C, B)
    sumsq = ar_tot[:, B:2 * B]
    cnt = GS * N
    mean_g = (gsel.T.astype(np.float64) @ sums).T / cnt          # (B, G)
    ex2_g = (gsel.T.astype(np.float64) @ sumsq).T / cnt
    var_g = ex2_g - mean_g ** 2
    r_g = 1.0 / np.sqrt(var_g + EPS)

    a = r_g[:, cg] * np.asarray(gn_g, np.float64)[None, :]       # (B, C)
    d = np.asarray(gn_b, np.float64)[None, :] - mean_g[:, cg] * a
    sv = s_tot.reshape(C, B, NH).transpose(1, 2, 0)              # (B, NH, C)
    zv = z_tot.reshape(B, NH)
    y = a[:, None, :] * (sv / zv[:, :, None]) + d[:, None, :]    # (B, NH, C)

    Wvr = np.asarray(Wv, np.float64).reshape(NH, HD, C)
    o1 = np.einsum("hdc,bhc->bhd", Wvr, y).reshape(B, C) + np.asarray(bv, np.float64)
    out = o1 @ np.asarray(Wo, np.float64).T + np.asarray(bo, np.float64)
    return out.astype(np.float32)
